# revision 81
# baseline (speedup 1.0000x reference)
"""MLA (multi-head latent attention) forward kernel for Trainium2, 8 NeuronCores.

Sharding: 8 cores = 2 (batch) x 4 (head-groups of 10 heads).
Each core, for its batch b and its 10 heads:
  - computes 1/4 of the fused down-projection a = x @ w_a (sequence-sharded
    within the batch group, transposed-activation layout), rmsnorm + k_pe rope
    on its slice, AllGather of the normalized kv latents across the 4 cores
  - q up-projection ONLY for its own 512 positions (from its local aq in
    SBUF -- the q latents are never gathered); the resulting qT head-chunks
    are AllGathered in 3 pipelined collectives so attention on early head
    chunks overlaps later gathers
  - kv up-projection for its heads over all positions, causal attention, and
    the partial o-projection (w_o rows of its heads).  Host sums the 4
    partials per batch.

Device layout notes:
  - activations are kept transposed ([feature, seq]) so weights act as the
    stationary lhsT operand of the PE in their natural [in, out] orientation.
  - attention computes scoresT [keys, q]; softmax runs without max-subtraction
    (scores are bounded by construction), masking is a binary multiply on the
    exp'd probabilities.  AV is computed transposed (oT[dv,q] = sum_kt
    v_kt^T @ probsT_kt) accumulating straight into PSUM, so no per-qt PE
    transposes are needed.  The softmax denominator comes from bf16 DVE sums
    of the probsT tiles over key-tiles followed by a single ones-matmul
    partition reduction per (head, q-chunk); 1/den is broadcast with a
    rank-1 matmul and applied on PSUM eviction.
"""

import math
import sys
from dataclasses import dataclass

if "/opt/trn_rl_repo" not in sys.path:
    sys.path.insert(0, "/opt/trn_rl_repo")

import ml_dtypes
import numpy as np

BF16 = ml_dtypes.bfloat16


@dataclass(frozen=True)
class Cfg:
    HID: int = 5120
    S: int = 2048
    QLR: int = 1536
    KVLR: int = 512
    DN: int = 128
    DR: int = 64
    DV: int = 128
    HPC: int = 10          # heads per core
    CHUNK: int = 512       # q-position chunk (PSUM bank width)
    GS: int = 1            # cores per batch group (sequence-shard of phase A)
    NCORES: int = 8
    EPS: float = 1e-6
    THETA: float = 10000.0

    @property
    def DQK(self):
        return self.DN + self.DR

    @property
    def PEH(self):
        return self.DR // 2

    @property
    def SL(self):
        return self.S // self.GS


FULL = Cfg(GS=4)

# head-chunks for the pipelined qT gathers: (m-tile list, head list).
# m-tiles 0..9 are per-head nope tiles; 10..14 are pe tiles (2 heads each).


def _chunks(c: Cfg):
    MT_QN = c.HPC
    out = []
    bounds = [0, 2, 4, 7, c.HPC]
    pe_done = 0
    for h0, h1 in zip(bounds, bounds[1:]):
        heads = list(range(h0, h1))
        # pe m-tiles first: their rope (DVE) tail then overlaps the nope
        # matmuls, so the chunk's spills finish with the matmuls.  A pe
        # m-tile goes with the chunk containing its FIRST head (an odd
        # boundary head reads its pe rows from the previous chunk).
        pe_hi = (h1 + 1) // 2
        mts = [MT_QN + j for j in range(pe_done, pe_hi)] + heads
        pe_done = pe_hi
        out.append((mts, heads))
    return out


def build_program(c: Cfg, stop_after: str = "E", debug: bool = False):
    import contextlib

    import concourse.bass as bass  # noqa: F401
    import concourse.mybir as mybir
    import concourse.tile as tile
    from concourse import bacc

    dt = mybir.dt
    BF = dt.bfloat16
    F32 = dt.float32
    Alu = mybir.AluOpType
    Act = mybir.ActivationFunctionType

    KT_HID = c.HID // 128
    KT_Q = c.QLR // 128
    KT_KV = c.KVLR // 128
    NQC = c.S // c.CHUNK
    GS = c.GS
    SL = c.SL
    NLC = SL // c.CHUNK             # local q-chunks in phase A
    ST = c.S // 128
    H = c.HPC
    TPC = c.CHUNK // 128            # 128-tiles per chunk (4)
    MT_QN = H * c.DN // 128
    MT_QP = H * c.DR // 128
    KROWS = H * c.DN
    VCOLS = H * c.DV
    MT_O = c.HID // 128
    SCALE = 1.0 / math.sqrt(c.DQK)
    CHUNKS = _chunks(c)

    assert c.DN == 128 and c.DV == 128 and c.DR == 64 and H % 2 == 0
    assert SL % c.CHUNK == 0
    _PH = ["A", "C", "B", "D", "E"]
    enabled = set(_PH[:_PH.index(stop_after) + 1])

    nc = bacc.Bacc("TRN2", num_devices=(c.NCORES if GS > 1 else None))
    MT_A = KT_Q + KT_KV + 1
    xT = nc.dram_tensor("xT", [128, KT_HID * SL], BF, kind="ExternalInput")
    w_a = nc.dram_tensor("w_a", [128, MT_A * KT_HID * 128], BF,
                         kind="ExternalInput")
    # full w_qb (all GS head-groups): every core computes q for ALL heads at
    # its own positions, then AllToAll delivers each core its heads' rows
    w_qb = nc.dram_tensor("w_qb", [128, GS * (MT_QN + MT_QP) * KT_Q * 128],
                          BF, kind="ExternalInput")
    w_kvb = nc.dram_tensor("w_kvb", [128, KT_KV * (KROWS + VCOLS)], BF,
                           kind="ExternalInput")
    w_o = nc.dram_tensor("w_o", [128, MT_O * H * 128], BF,
                         kind="ExternalInput")
    cosA = nc.dram_tensor("cosA", [128, SL], BF, kind="ExternalInput")
    sinA = nc.dram_tensor("sinA", [128, SL], BF, kind="ExternalInput")
    lnkv = nc.dram_tensor("lnkv", [128, KT_KV], F32, kind="ExternalInput")
    maskm = nc.dram_tensor("maskm", [128, TPC, c.CHUNK], BF, kind="ExternalInput")
    outT = nc.dram_tensor("outT", [c.HID, c.S], F32, kind="ExternalOutput")
    DBG = "ExternalOutput" if debug else "Internal"
    aglkv = nc.dram_tensor("aglkv", [(KT_KV + 1) * 128, SL], BF, kind="Internal")
    # qT exchange buffers for the ReduceScatter-emulated all-to-all:
    # [dst_group, src_rank, mt, 128, SL].  Host pre-zeroes them
    # (ExternalInput); each core writes only its own src_rank slots (via a
    # cc_rank dynamic offset), so the add-reduce concatenates rank blocks.
    aglqc = [nc.dram_tensor(f"aglqc{i}",
                            [GS * GS * len(mts) * 128, SL], BF,
                            kind="Internal")
             for i, (mts, _) in enumerate(CHUNKS)]
    if debug:
        qdbg = [nc.dram_tensor(f"qdbg{i}", [GS * GS * len(mts) * 128, SL],
                               BF, kind="ExternalOutput")
                for i, (mts, _) in enumerate(CHUNKS)]
        qadbg = [nc.dram_tensor(f"qadbg{i}", [GS * len(mts) * 128, SL], BF,
                                kind="ExternalOutput")
                 for i, (mts, _) in enumerate(CHUNKS)]
        qn_d = nc.dram_tensor("qn_d", [128, c.S], BF, kind="ExternalOutput")
        knope_d = nc.dram_tensor("knope_d", [128, H * c.S], BF,
                                 kind="ExternalOutput")
        vv_d = nc.dram_tensor("vv_d", [128, ST * H * c.DV], BF,
                              kind="ExternalOutput")
        oT_d = nc.dram_tensor("oT_d", [128, H * c.S], BF,
                              kind="ExternalOutput")
        kpe_d = nc.dram_tensor("kpe_d", [c.DR, c.S], BF,
                               kind="ExternalOutput")
    if GS > 1:
        aggkv = nc.dram_tensor("aggkv", [GS * (KT_KV + 1) * 128, SL], BF,
                               kind="Internal")
        aggqc = [nc.dram_tensor(f"aggqc{i}", [GS * len(mts) * 128, SL], BF,
                                kind="Internal")
                 for i, (mts, _) in enumerate(CHUNKS)]
    else:
        aggkv = aglkv
        aggqc = aglqc  # GS=1: same [1*1*nmt*128, SL] shape

    xT_r = xT.ap().rearrange("p (t s) -> p t s", s=SL)
    w_a_r = w_a.ap().rearrange("p (mt k m) -> p mt (k m)", mt=MT_A, m=128)
    w_qb_r = w_qb.ap().rearrange("p (mt k m) -> p mt (k m)",
                                 mt=GS * (MT_QN + MT_QP), m=128)
    w_kvb_r = w_kvb.ap().rearrange("p (k m) -> p k m", k=KT_KV)
    w_o_r = w_o.ap().rearrange("p (mt k m) -> p mt (k m)", mt=MT_O, m=128)
    aglkv_r = aglkv.ap().rearrange("(t p) s -> p t s", p=128)
    aggkv_r = aggkv.ap().rearrange("(g t p) s -> g p t s", g=GS, p=128)
    aglqc_r = [a.ap().rearrange("(g t p) s -> g t p s", g=GS, p=128)
               for a in aglqc] if GS == 1 else None
    aggqc_r = [a.ap().rearrange("(g t p) s -> g t p s", g=GS, p=128)
               for a in aggqc]
    outT_ap = outT.ap()

    def emit_rope(nc, pool, dst64, src64, cos_ap, sin_ap, W, p0=0):
        # cos_ap/sin_ap are [128, W] (table replicated every PEH partitions);
        # slices are taken at each operand's base partition because DVE
        # tensor_tensor requires equal base partitions for SBUF inputs.
        ph = c.PEH
        t1, t2 = src64[0:ph], src64[ph:2 * ph]
        d1, d2 = dst64[0:ph], dst64[ph:2 * ph]
        c1, s1 = cos_ap[p0:p0 + ph], sin_ap[p0:p0 + ph]
        c2, s2 = cos_ap[p0 + ph:p0 + 2 * ph], sin_ap[p0 + ph:p0 + 2 * ph]
        ra = pool.tile([ph, W], F32, tag="rope_a", name="rope_a")
        rb = pool.tile([ph, W], F32, tag="rope_b", name="rope_b")
        nc.vector.tensor_tensor(out=ra, in0=t1, in1=c1, op=Alu.mult)
        nc.vector.tensor_tensor(out=rb, in0=t2, in1=s2, op=Alu.mult)
        nc.vector.tensor_tensor(out=d1, in0=ra, in1=rb, op=Alu.subtract)
        nc.vector.tensor_tensor(out=ra, in0=t2, in1=c2, op=Alu.mult)
        nc.vector.tensor_tensor(out=rb, in0=t1, in1=s1, op=Alu.mult)
        nc.vector.tensor_tensor(out=d2, in0=ra, in1=rb, op=Alu.add)

    with tile.TileContext(nc, pool_alloc_mode="queue") as tc:
        with contextlib.ExitStack() as top:
            pers = top.enter_context(tc.tile_pool(name="pers", bufs=1))
            cosa_sb = pers.tile([128, SL], BF, tag="cosa_sb")
            sina_sb = pers.tile([128, SL], BF, tag="sina_sb")
            lnkv_sb = pers.tile([128, KT_KV], F32, tag="lnkv_sb")
            mask_sb = pers.tile([128, TPC, c.CHUNK], BF, tag="mask_sb")
            ones_f = pers.tile([1, 128], F32, tag="ones_f")
            ones_c = pers.tile([128, 1], BF, tag="ones_c")
            eps_sb = pers.tile([1, 1], F32, tag="eps_sb")
            eps2_sb = pers.tile([1, 1], F32, tag="eps2_sb")
            kpe = pers.tile([c.DR, c.S], BF, tag="kpe")
            # gathered kv latents, loaded early via the Pool queue; lives
            # through phase D (B's matmuls and nothing later read it, but
            # the pool is top-level for LIFO simplicity)
            pakv = top.enter_context(tc.tile_pool(name="pakv", bufs=1))
            akv_f = pakv.tile([128, KT_KV, c.S], BF, tag="akv_f")

            with contextlib.ExitStack() as st_ac:
                paq = st_ac.enter_context(tc.tile_pool(name="paq", bufs=1))
                aq_c = paq.tile([128, KT_Q, SL], BF, tag="aq_c")
                rnr_q = paq.tile([1, SL], F32, tag="rnr_q")
                bcq_sb = paq.tile([128, SL], F32, tag="bcq_sb")
                zero_sb = paq.tile([128, 8, 512], BF, tag="zero_sb")
                pcb = st_ac.enter_context(
                    tc.tile_pool(name="pcb", bufs=1, space="PSUM"))
                bcq = pcb.tile([128, SL], F32, tag="bcq")

                # ---- phase A: local a-proj + rmsnorm + k_pe rope + kv gather
                with contextlib.ExitStack() as st:
                    pax = st.enter_context(tc.tile_pool(name="pax", bufs=1))
                    paw = st.enter_context(tc.tile_pool(name="paw", bufs=2))
                    pat = st.enter_context(tc.tile_pool(name="pat", bufs=2))
                    paps = st.enter_context(
                        tc.tile_pool(name="paps", bufs=3, space="PSUM"))
                    pssq = st.enter_context(
                        tc.tile_pool(name="pssq", bufs=1, space="PSUM"))
                    pbc = st.enter_context(
                        tc.tile_pool(name="pbc", bufs=2, space="PSUM"))

                    mtiles = ([("kv", i) for i in range(KT_KV)]
                              + [("pe", 0)]
                              + [("q", i) for i in range(KT_Q)])
                    groups = [[b * GS + j for j in range(GS)]
                              for b in range(c.NCORES // GS)]
                    x_sb = pax.tile([128, KT_HID, SL], BF, tag="x_sb")
                    # first weight tile ahead of everything, then x in chunks
                    # so A's first matmuls start as early as possible
                    wa_first = paw.tile([128, KT_HID, 128], BF, tag="wa_sb",
                                        name="wa_sb")
                    nc.sync.dma_start(
                        out=wa_first,
                        in_=w_a_r[:, 0].rearrange("p (k m) -> p k m", m=128))
                    for xh in range(4):
                        ksl = slice(xh * (KT_HID // 4), (xh + 1) * (KT_HID // 4))
                        nc.sync.dma_start(out=x_sb[:, ksl, :], in_=xT_r[:, ksl, :])
                    nc.sync.dma_start(out=cosa_sb, in_=cosA.ap())
                    nc.sync.dma_start(out=sina_sb, in_=sinA.ap())
                    nc.sync.dma_start(out=lnkv_sb, in_=lnkv.ap())
                    nc.sync.dma_start(out=mask_sb, in_=maskm.ap())
                    nc.vector.memset(eps_sb, c.EPS)
                    nc.vector.memset(eps2_sb, c.EPS / (SCALE * SCALE))
                    nc.vector.memset(ones_f, 1.0)
                    nc.vector.memset(ones_c, 1.0)
                    akv_c = pat.tile([128, KT_KV, SL], BF, tag="akv_c", bufs=1)
                    ssq_q = pssq.tile([1, SL], F32, tag="ssq_q")
                    ssq_kv = pssq.tile([1, SL], F32, tag="ssq_kv")
                    kperaw = pat.tile([c.DR, SL], BF, tag="kperaw", bufs=1)
                    kpel = pat.tile([c.DR, SL], BF, tag="kpel", bufs=1)

                    def normalize(ssq, ln_sb, ktn, denom, dst):
                        for qi in range(NLC):
                            cs = slice(qi * c.CHUNK, (qi + 1) * c.CHUNK)
                            rn = pat.tile([1, c.CHUNK], F32, tag="rn", name="rn")
                            nc.scalar.activation(
                                rn, ssq[:, cs], Act.Sqrt,
                                bias=eps_sb, scale=1.0 / denom)
                            rnr = pat.tile([1, c.CHUNK], F32, tag="rnr",
                                           name="rnr")
                            nc.vector.reciprocal(rnr, rn)
                            bc = pbc.tile([128, c.CHUNK], F32, tag="bc",
                                          name="bc")
                            nc.tensor.matmul(
                                bc, ones_f, rnr, start=True, stop=True)
                            for t in range(ktn):
                                tgt = dst[:, t, cs]
                                nc.vector.scalar_tensor_tensor(
                                    out=tgt, in0=tgt,
                                    scalar=ln_sb[:, t:t + 1], in1=bc,
                                    op0=Alu.mult, op1=Alu.mult)

                    def emit_kv_part():
                        # normalize kv + rope k_pe, spill, and gather — emitted
                        # before the q m-tiles so the collective overlaps them
                        normalize(ssq_kv, lnkv_sb, KT_KV, c.KVLR, akv_c)
                        for qi in range(NLC):
                            cs = slice(qi * c.CHUNK, (qi + 1) * c.CHUNK)
                            emit_rope(nc, pat, kpel[:, cs], kperaw[:, cs],
                                      cosa_sb[:, cs], sina_sb[:, cs], c.CHUNK)
                        nc.sync.dma_start(out=aglkv_r[:, 0:KT_KV, :], in_=akv_c)
                        nc.sync.dma_start(out=aglkv_r[0:c.DR, KT_KV, :], in_=kpel)
                        if GS > 1:
                            nc.gpsimd.collective_compute(
                                "AllGather", mybir.AluOpType.bypass,
                                replica_groups=groups,
                                ins=[aglkv.ap()], outs=[aggkv.ap()])

                    for mti, (seg, ti) in enumerate(mtiles):
                        mw = c.DR if seg == "pe" else 128
                        if mti == 0:
                            wa_sb = wa_first
                        else:
                            wa_sb = paw.tile([128, KT_HID, 128], BF,
                                             tag="wa_sb", name="wa_sb")
                            nc.sync.dma_start(
                                out=wa_sb,
                                in_=w_a_r[:, mti].rearrange("p (k m) -> p k m",
                                                            m=128))
                        for qi in range(NLC):
                            cs = slice(qi * c.CHUNK, (qi + 1) * c.CHUNK)
                            ps = paps.tile([128, c.CHUNK], F32, tag="aps",
                                           name="ps")
                            for kt in range(KT_HID):
                                nc.tensor.matmul(
                                    ps[:mw], wa_sb[:, kt, :mw],
                                    x_sb[:, kt, cs],
                                    start=(kt == 0), stop=(kt == KT_HID - 1))
                            if seg == "q":
                                nc.scalar.copy(aq_c[:, ti, cs], ps)
                                sq = pat.tile([128, c.CHUNK], BF, tag="sq",
                                              bufs=3, name="sq")
                                nc.scalar.square(sq, ps)
                                nc.tensor.matmul(
                                    ssq_q[:, cs], ones_c, sq,
                                    start=(ti == 0), stop=(ti == KT_Q - 1))
                            elif seg == "kv":
                                nc.scalar.copy(akv_c[:, ti, cs], ps)
                                sq = pat.tile([128, c.CHUNK], BF, tag="sq",
                                              bufs=3, name="sq")
                                nc.scalar.square(sq, ps)
                                nc.tensor.matmul(
                                    ssq_kv[:, cs], ones_c, sq,
                                    start=(ti == 0), stop=(ti == KT_KV - 1))
                            else:
                                nc.scalar.copy(kperaw[:, cs], ps[:mw])
                        if seg == "pe":
                            emit_kv_part()
                    # q-side rmsnorm: the ln weight is folded into w_qb on
                    # the host; 1/rms (with the attention scale folded in)
                    # is applied per column at C's PSUM eviction, so C's
                    # matmuls need no normalization barrier at all.
                    rn = pat.tile([1, SL], F32, tag="rn", name="rn")
                    nc.scalar.activation(rn, ssq_q, Act.Sqrt, bias=eps2_sb,
                                         scale=1.0 / (c.QLR * SCALE * SCALE))
                    nc.vector.reciprocal(rnr_q, rn)
                    nc.tensor.matmul(bcq, ones_f, rnr_q, start=True, stop=True)
                    # tensor_tensor cannot take two PSUM operands: stage the
                    # broadcast through SBUF once
                    nc.scalar.copy(bcq_sb, bcq)

                # ---- phase C: local q up-projection + pipelined qT exchange
                if "C" in enabled:
                    from concourse.bass import ds
                    if GS > 1:
                        rank_sv = nc.sync.cc_rank(groups)
                        # zero the exchange buffers (the spills then only
                        # overwrite this core's own src-rank slots); issued
                        # from the Act queue to keep SP free
                        nc.vector.memset(zero_sb, 0.0)
                        for ci, (mts, _) in enumerate(CHUNKS):
                            nblk = GS * GS * len(mts)
                            az = aglqc[ci].ap().rearrange(
                                "(n p) s -> p n s", p=128)
                            for j in range(0, nblk, 8):
                                jn = min(8, nblk - j)
                                nc.scalar.dma_start(
                                    out=az[:, j:j + jn, :],
                                    in_=zero_sb[:, 0:jn, :])
                    with contextlib.ExitStack() as st:
                        pcw = st.enter_context(tc.tile_pool(name="pcw", bufs=3))
                        pce = st.enter_context(tc.tile_pool(name="pce", bufs=3))
                        pcps = st.enter_context(
                            tc.tile_pool(name="pcps", bufs=4, space="PSUM"))
                        for ci, (mts, heads) in enumerate(CHUNKS):
                          for grp in range(GS):
                            for pos, mt in enumerate(mts):
                                mtg = grp * (MT_QN + MT_QP) + mt
                                wq_sb = pcw.tile([128, KT_Q, 128], BF, tag="wq")
                                nc.sync.dma_start(
                                    out=wq_sb,
                                    in_=w_qb_r[:, mtg].rearrange(
                                        "p (k m) -> p k m", m=128))
                                for qi in range(NLC):
                                    cs = slice(qi * c.CHUNK, (qi + 1) * c.CHUNK)
                                    ps = pcps.tile([128, c.CHUNK], F32,
                                                   tag="qps")
                                    for kt in range(KT_Q):
                                        nc.tensor.matmul(
                                            ps, wq_sb[:, kt, :],
                                            aq_c[:, kt, cs],
                                            start=(kt == 0),
                                            stop=(kt == KT_Q - 1))
                                    qsb = pce.tile([128, c.CHUNK], BF,
                                                   tag="qsb")
                                    nc.vector.tensor_tensor(
                                        out=qsb, in0=ps, in1=bcq_sb[:, cs],
                                        op=Alu.mult)
                                    if mt >= MT_QN:
                                        # pe tiles use the half-grouped
                                        # layout [t1_h0 t1_h1 t2_h0 t2_h1]
                                        # so one rope pass covers 2 heads
                                        roped = pce.tile([128, c.CHUNK], BF,
                                                         tag="roped")
                                        # SBUF tensor_tensor inputs must
                                        # share a start partition: slice the
                                        # (32-periodic) tables at each
                                        # operand's base
                                        t1, t2 = qsb[0:64], qsb[64:128]
                                        d1, d2 = roped[0:64], roped[64:128]
                                        co0 = cosa_sb[0:64, cs]
                                        si0 = sina_sb[0:64, cs]
                                        co6 = cosa_sb[64:128, cs]
                                        si6 = sina_sb[64:128, cs]
                                        ra = pce.tile([64, c.CHUNK], F32,
                                                      tag="rra")
                                        rb = pce.tile([64, c.CHUNK], F32,
                                                      tag="rrb")
                                        nc.vector.tensor_tensor(
                                            out=ra, in0=t1, in1=co0,
                                            op=Alu.mult)
                                        nc.vector.tensor_tensor(
                                            out=rb, in0=t2, in1=si6,
                                            op=Alu.mult)
                                        nc.vector.tensor_tensor(
                                            out=d1, in0=ra, in1=rb,
                                            op=Alu.subtract)
                                        nc.vector.tensor_tensor(
                                            out=ra, in0=t2, in1=co6,
                                            op=Alu.mult)
                                        nc.vector.tensor_tensor(
                                            out=rb, in0=t1, in1=si0,
                                            op=Alu.mult)
                                        nc.vector.tensor_tensor(
                                            out=d2, in0=ra, in1=rb,
                                            op=Alu.add)
                                        qsb = roped
                                    nmt = len(mts)
                                    if GS > 1:
                                        # row block [dst=grp][src=rank][pos]
                                        off = (rank_sv * (nmt * 128)
                                               + (grp * GS * nmt + pos) * 128)
                                        nc.sync.dma_start(
                                            out=aglqc[ci].ap()[
                                                ds(off, 128), cs],
                                            in_=qsb)
                                    else:
                                        nc.sync.dma_start(
                                            out=aglqc_r[ci][0, pos][:, cs],
                                            in_=qsb)
                            if debug:
                                nc.sync.dma_start(out=qdbg[ci].ap(),
                                                  in_=aglqc[ci].ap())
                            if GS > 1:
                                nc.gpsimd.collective_compute(
                                    "ReduceScatter", mybir.AluOpType.add,
                                    replica_groups=groups,
                                    ins=[aglqc[ci].ap()], outs=[aggqc[ci].ap()])
                            if ci == 0:
                                # kpe + gathered-kv loads wait on the
                                # kv-gather: issue from the Pool queue after
                                # gather-c1 so neither the SP queue nor the
                                # first qT gather is blocked behind them
                                for g in range(GS):
                                    nc.gpsimd.dma_start(
                                        out=kpe[:, g * SL:(g + 1) * SL],
                                        in_=aggkv_r[g, 0:c.DR, KT_KV, :])
                                    nc.gpsimd.dma_start(
                                        out=akv_f[:, :, g * SL:(g + 1) * SL],
                                        in_=aggkv_r[g, :, 0:KT_KV, :])

            # ------------- phase B: kv up-projection -------------------------
            if "B" in enabled:
                pkv = top.enter_context(tc.tile_pool(name="pkv", bufs=1))
                knope = pkv.tile([128, H, c.S], BF, tag="knope")
                vv = pkv.tile([128, ST, H, c.DV], BF, tag="vv")

                with contextlib.ExitStack() as st:
                    pbw = st.enter_context(tc.tile_pool(name="pbw", bufs=1))
                    pbps = st.enter_context(
                        tc.tile_pool(name="pbps", bufs=4, space="PSUM"))
                    wkv_sb = pbw.tile([128, KT_KV, KROWS + VCOLS], BF, tag="wkv")
                    # split load: the knope matmuls only need the k half
                    nc.sync.dma_start(out=wkv_sb[:, :, 0:KROWS],
                                      in_=w_kvb_r[:, :, 0:KROWS])
                    nc.sync.dma_start(out=wkv_sb[:, :, KROWS:],
                                      in_=w_kvb_r[:, :, KROWS:])
                    for mt in range(H):
                        for qc in range(NQC):
                            ps = pbps.tile([128, c.CHUNK], F32, tag="kps")
                            for kt in range(KT_KV):
                                nc.tensor.matmul(
                                    ps, wkv_sb[:, kt, mt * 128:(mt + 1) * 128],
                                    akv_f[:, kt, qc * c.CHUNK:(qc + 1) * c.CHUNK],
                                    start=(kt == 0), stop=(kt == KT_KV - 1))
                            nc.scalar.copy(
                                knope[:, mt, qc * c.CHUNK:(qc + 1) * c.CHUNK], ps)
                    vch = []
                    v0 = 0
                    while v0 < VCOLS:
                        vw = min(512, VCOLS - v0)
                        vch.append((v0, vw))
                        v0 += vw
                    for stt in range(ST):
                        for v0, vw in vch:
                            ps = pbps.tile([128, 512], F32, tag="vps")
                            for kt in range(KT_KV):
                                nc.tensor.matmul(
                                    ps[:, :vw],
                                    akv_f[:, kt, stt * 128:(stt + 1) * 128],
                                    wkv_sb[:, kt, KROWS + v0:KROWS + v0 + vw],
                                    start=(kt == 0), stop=(kt == KT_KV - 1))
                            h0, hn = v0 // c.DV, vw // c.DV
                            nc.scalar.copy(
                                vv[:, stt, h0:h0 + hn, :],
                                ps[:, :vw].rearrange("p (h d) -> p h d",
                                                     d=c.DV))

            # ---------------- phase D: attention -----------------------------
            if "D" in enabled:
                pot = top.enter_context(tc.tile_pool(name="pot", bufs=1))
                oT = pot.tile([128, H, c.S], BF, tag="oT")

                with contextlib.ExitStack() as st:
                    pdp = st.enter_context(tc.tile_pool(name="pdp", bufs=2))
                    pdq = st.enter_context(tc.tile_pool(name="pdq", bufs=2))
                    pda = st.enter_context(tc.tile_pool(name="pda", bufs=2))
                    pde = st.enter_context(tc.tile_pool(name="pde", bufs=2))
                    pds = st.enter_context(
                        tc.tile_pool(name="pds", bufs=3, space="PSUM"))
                    pdo = st.enter_context(
                        tc.tile_pool(name="pdo", bufs=2, space="PSUM"))
                    pdd = st.enter_context(
                        tc.tile_pool(name="pdd", bufs=1, space="PSUM"))

                    def emit_evict(po, acc, h, cs):
                        # softmax denominator (partition reduction of the
                        # DVE kt-sums) + broadcast of 1/den + PSUM eviction.
                        # Deferred one (h, qc) iteration so the PE queue
                        # never waits on the Act/DVE tail of the current one.
                        den = pdd.tile([1, c.CHUNK], F32, tag="den")
                        nc.tensor.matmul(den, ones_c, acc, start=True,
                                         stop=True)
                        rec = pde.tile([1, c.CHUNK], F32, tag="rec")
                        nc.vector.reciprocal(rec, den)
                        bcd = pdd.tile([128, c.CHUNK], F32, tag="bcd")
                        nc.tensor.matmul(bcd, ones_f, rec, start=True,
                                         stop=True)
                        bcd_sb = pde.tile([128, c.CHUNK], F32, tag="bcd_sb")
                        nc.scalar.copy(bcd_sb, bcd)
                        nc.vector.tensor_tensor(
                            out=oT[:, h, cs], in0=po, in1=bcd_sb,
                            op=Alu.mult)

                    pending = None
                    # zero the score psum buffers once: the triangular
                    # diagonal matmuls leave stale sub-regions that are
                    # exp'd then masked; first use must not see garbage
                    for _ in range(3):
                        zt = pds.tile([128, c.CHUNK], F32, tag="sc")
                        nc.vector.memset(zt, 0.0)
                    mtloc = {mt: (ci, i)
                             for ci, (mts, _) in enumerate(CHUNKS)
                             for i, mt in enumerate(mts)}
                    for ci, (mts, heads) in enumerate(CHUNKS):
                        for h in heads:
                            cn, i_n = mtloc[h]
                            cp, i_p = mtloc[MT_QN + h // 2]
                            # half-grouped pe layout: head h's rope halves
                            # sit at rows [32*(h%2)] and [64 + 32*(h%2)]
                            ph = c.PEH
                            h1 = (h % 2) * ph
                            h2 = 64 + (h % 2) * ph
                            qn = pdq.tile([128, c.S], BF, tag="qn")
                            qp = pdq.tile([c.DR, c.S], BF, tag="qp")
                            if GS > 1:
                                for g in range(GS):
                                    gsl = slice(g * SL, (g + 1) * SL)
                                    nc.sync.dma_start(
                                        out=qn[:, gsl],
                                        in_=aggqc_r[cn][g, i_n])
                                    nc.sync.dma_start(
                                        out=qp[0:ph, gsl],
                                        in_=aggqc_r[cp][g, i_p,
                                                        h1:h1 + ph])
                                    nc.sync.dma_start(
                                        out=qp[ph:2 * ph, gsl],
                                        in_=aggqc_r[cp][g, i_p,
                                                        h2:h2 + ph])
                            else:
                                nc.sync.dma_start(out=qn,
                                                  in_=aglqc_r[cn][0, i_n])
                                nc.sync.dma_start(
                                    out=qp[0:ph, :],
                                    in_=aglqc_r[cp][0, i_p, h1:h1 + ph])
                                nc.sync.dma_start(
                                    out=qp[ph:2 * ph, :],
                                    in_=aglqc_r[cp][0, i_p, h2:h2 + ph])
                            if debug and h == heads[0] and ci == 0:
                                nc.sync.dma_start(out=qn_d.ap(), in_=qn)
                            for qc in range(NQC):
                                col = qc * c.CHUNK
                                cs = slice(col, col + c.CHUNK)
                                kmax = min(TPC * qc + TPC, ST)
                                probs = pdp.tile([128, ST, c.CHUNK], BF,
                                                 tag="probs")
                                po = pdo.tile([128, c.CHUNK], F32, tag="po")
                                acc = pda.tile([128, c.CHUNK], BF, tag="acc")
                                av_pend = []
                                for kt in range(kmax):
                                    ps = pds.tile([128, c.CHUNK], F32,
                                                  tag="sc")
                                    # diagonal tiles: only compute the
                                    # causal q-columns; the stale region is
                                    # exp'd (bounded) then masked to zero
                                    q0 = max(0, (kt - TPC * qc) * 128)
                                    nc.tensor.matmul(
                                        ps[:, q0:],
                                        knope[:, h, kt * 128:(kt + 1) * 128],
                                        qn[:, col + q0:col + c.CHUNK],
                                        start=True, stop=False)
                                    nc.tensor.matmul(
                                        ps[:, q0:],
                                        kpe[:, kt * 128:(kt + 1) * 128],
                                        qp[:, col + q0:col + c.CHUNK],
                                        start=False, stop=True)
                                    nc.scalar.activation(
                                        probs[:, kt, :], ps, Act.Exp)
                                    d = kt - TPC * qc
                                    if d >= 0:
                                        nc.vector.tensor_tensor(
                                            out=probs[:, kt, :],
                                            in0=probs[:, kt, :],
                                            in1=mask_sb[:, d, :], op=Alu.mult)
                                    # AV deferred two kt so PE never waits
                                    # on exp/mask of a just-computed tile.
                                    # Diagonal tiles only touch the causal
                                    # q-columns (the rest stays accumulated
                                    # from earlier kt only — exactly causal)
                                    av_pend.append(kt)
                                    if len(av_pend) > 2:
                                        j = av_pend.pop(0)
                                        jq = max(0, (j - TPC * qc) * 128)
                                        nc.tensor.matmul(
                                            po[:, jq:], vv[:, j, h, :],
                                            probs[:, j, jq:],
                                            start=(j == 0), stop=False)
                                    # denominator kt-sums ride along on DVE
                                    if kt == 1:
                                        nc.vector.tensor_tensor(
                                            out=acc, in0=probs[:, 0, :],
                                            in1=probs[:, 1, :], op=Alu.add)
                                    elif kt >= 2:
                                        nc.vector.tensor_tensor(
                                            out=acc, in0=acc,
                                            in1=probs[:, kt, :], op=Alu.add)
                                for n_, j in enumerate(av_pend):
                                    jq = max(0, (j - TPC * qc) * 128)
                                    nc.tensor.matmul(
                                        po[:, jq:], vv[:, j, h, :],
                                        probs[:, j, jq:],
                                        start=(j == 0),
                                        stop=(n_ == len(av_pend) - 1))
                                if pending is not None:
                                    emit_evict(*pending)
                                pending = (po, acc, h, cs)
                    emit_evict(*pending)

            if debug and "D" in enabled:
                for ci in range(len(CHUNKS)):
                    nc.sync.dma_start(out=qadbg[ci].ap(),
                                      in_=aggqc[ci].ap())
                nc.sync.dma_start(
                    out=knope_d.ap().rearrange("p (h s) -> p h s", s=c.S),
                    in_=knope)
                nc.sync.dma_start(
                    out=vv_d.ap().rearrange("p (a b d) -> p a b d",
                                            b=H, d=c.DV),
                    in_=vv)
                nc.sync.dma_start(
                    out=oT_d.ap().rearrange("p (h s) -> p h s", s=c.S),
                    in_=oT)
                nc.sync.dma_start(out=kpe_d.ap(), in_=kpe)

            # ---------------- phase E: o-projection --------------------------
            if "E" in enabled:
                with contextlib.ExitStack() as st:
                    pew = st.enter_context(tc.tile_pool(name="pew", bufs=3))
                    peo = st.enter_context(tc.tile_pool(name="peo", bufs=3))
                    peps = st.enter_context(
                        tc.tile_pool(name="peps", bufs=4, space="PSUM"))
                    for mt in range(MT_O):
                        wo_sb = pew.tile([128, H, 128], BF, tag="wo")
                        nc.sync.dma_start(
                            out=wo_sb,
                            in_=w_o_r[:, mt].rearrange(
                                "p (k m) -> p k m", m=128))
                        for qc in range(NQC):
                            col = qc * c.CHUNK
                            ps = peps.tile([128, c.CHUNK], F32, tag="ops")
                            for kt in range(H):
                                nc.tensor.matmul(
                                    ps, wo_sb[:, kt, :],
                                    oT[:, kt, col:col + c.CHUNK],
                                    start=(kt == 0), stop=(kt == H - 1))
                            ob = peo.tile([128, c.CHUNK], F32, tag="ob")
                            nc.scalar.copy(ob, ps)
                            nc.sync.dma_start(
                                out=outT_ap[mt * 128:(mt + 1) * 128,
                                            col:col + c.CHUNK],
                                in_=ob)

    nc.compile()
    return nc


# ---------------------------------------------------------------------------
# host-side input preparation
# ---------------------------------------------------------------------------

def prep_shared(c: Cfg, w_a, q_ln_w, kv_ln_w):
    KT_Q = c.QLR // 128
    KT_KV = c.KVLR // 128
    TPC = c.CHUNK // 128
    half = c.PEH
    inv_freq = 1.0 / (c.THETA ** (np.arange(half, dtype=np.float32) / half))
    ang = np.arange(c.S, dtype=np.float32)[:, None] * inv_freq[None, :]
    cosT = np.ascontiguousarray(
        np.tile(np.cos(ang).T, (128 // half, 1))).astype(BF16)
    sinT = np.ascontiguousarray(
        np.tile(np.sin(ang).T, (128 // half, 1))).astype(BF16)
    k_idx = np.arange(128)[:, None]
    q_idx = np.arange(c.CHUNK)[None, :]
    maskm = np.stack(
        [(k_idx <= q_idx - 128 * d) for d in range(TPC)], axis=1
    ).astype(BF16)
    # w_a tiled: [p, mt, kt, 128] with the pe m-tile zero-padded to 128 cols
    MT_A = KT_Q + KT_KV + 1
    KT_HID = c.HID // 128
    wa = np.asarray(w_a, np.float32)
    # m-tile order in the kernel: kv tiles, pe, then q tiles
    order = ([c.QLR + i * 128 for i in range(KT_KV)]
             + [c.QLR + c.KVLR]
             + [i * 128 for i in range(KT_Q)])
    tiles = []
    for m0 in order:
        t = np.zeros((c.HID, 128), np.float32)
        wsrc = wa[:, m0:m0 + 128]
        t[:, :wsrc.shape[1]] = wsrc
        tiles.append(t)
    wa_t = np.stack(tiles, axis=1)  # [HID, MT_A, 128]
    wa_t = wa_t.reshape(KT_HID, 128, MT_A, 128).transpose(1, 2, 0, 3)
    wa_t = np.ascontiguousarray(wa_t.reshape(128, MT_A * KT_HID * 128))
    return {
        "w_a": wa_t.astype(BF16),
        "lnkv": np.ascontiguousarray(
            kv_ln_w.reshape(KT_KV, 128).T).astype(np.float32),
        "maskm": np.ascontiguousarray(maskm),
    }, cosT, sinT


def prep_group(c: Cfg, heads, w_qb, w_kvb, w_o, n_heads_total, q_ln_w):
    """Reorganize the up-projection weights for one head group."""
    # the q-side rmsnorm weight is folded into w_qb; the kernel applies
    # only the per-position 1/rms factor at C's eviction
    w_qb = w_qb * q_ln_w[:, None]
    wq = w_qb.reshape(c.QLR, n_heads_total, c.DQK)[:, heads, :]
    # pe columns in half-grouped layout: per head pair (2j, 2j+1) the
    # 128-wide m-tile is [t1_2j, t1_2j+1, t2_2j, t2_2j+1] (t = rope halves)
    ph = c.PEH
    wpe = wq[:, :, c.DN:].reshape(c.QLR, -1, 2, 2, ph)  # [q, pair, h, t, ph]
    wpe = wpe.transpose(0, 1, 3, 2, 4)                  # [q, pair, t, h, ph]
    wq_g = np.concatenate(
        [wq[:, :, :c.DN].reshape(c.QLR, -1), wpe.reshape(c.QLR, -1)],
        axis=1)
    wkv = w_kvb.reshape(c.KVLR, n_heads_total, c.DN + c.DV)[:, heads, :]
    wkv_g = np.concatenate(
        [wkv[:, :, :c.DN].reshape(c.KVLR, -1),
         wkv[:, :, c.DN:].reshape(c.KVLR, -1)], axis=1)
    wo_g = w_o.reshape(n_heads_total, c.DV, c.HID)[heads].reshape(-1, c.HID)
    H = c.HPC
    KT_Q = c.QLR // 128
    KT_KV = c.KVLR // 128
    MT_QB = wq_g.shape[1] // 128
    MT_O = c.HID // 128
    # device layouts: [p, mt, kt, 128] flattened per partition
    wq_t = wq_g.reshape(KT_Q, 128, MT_QB, 128).transpose(1, 2, 0, 3)
    wq_t = np.ascontiguousarray(wq_t.reshape(128, MT_QB * KT_Q * 128))
    wkv_t = wkv_g.reshape(KT_KV, 128, wkv_g.shape[1]).transpose(1, 0, 2)
    wkv_t = np.ascontiguousarray(wkv_t.reshape(128, -1))
    wo_t = wo_g.reshape(H, 128, MT_O, 128).transpose(1, 2, 0, 3)
    wo_t = np.ascontiguousarray(wo_t.reshape(128, MT_O * H * 128))
    return {
        "wq_part": wq_t.astype(BF16),
        "w_kvb": wkv_t.astype(BF16),
        "w_o": wo_t.astype(BF16),
    }


_PROGRAM = None


def _get_program():
    global _PROGRAM
    if _PROGRAM is None:
        _PROGRAM = build_program(FULL)
    return _PROGRAM


def kernel(x, w_a, q_ln_w, kv_ln_w, w_qb, w_kvb, w_o):
    from concourse.bass_utils import run_bass_kernel_spmd

    c = FULL
    x = np.asarray(x, dtype=np.float32)
    B = x.shape[0]
    n_heads = w_qb.shape[1] // c.DQK
    n_groups = n_heads // c.HPC
    assert B * n_groups == c.NCORES and n_groups == c.GS

    nc = _get_program()
    shared, cosT, sinT = prep_shared(c, np.asarray(w_a), np.asarray(q_ln_w),
                                     np.asarray(kv_ln_w))
    groups = [
        prep_group(c, slice(g * c.HPC, (g + 1) * c.HPC), np.asarray(w_qb),
                   np.asarray(w_kvb), np.asarray(w_o), n_heads,
                   np.asarray(q_ln_w))
        for g in range(n_groups)
    ]
    xTs = [np.ascontiguousarray(x[b].T).astype(BF16) for b in range(B)]

    # every core gets the FULL w_qb (group-major m-tiles)
    wq_full = np.ascontiguousarray(
        np.concatenate([g_["wq_part"] for g_ in groups], axis=1))
    in_maps = []
    for core in range(c.NCORES):
        b, g = divmod(core, n_groups)
        sl = slice(g * c.SL, (g + 1) * c.SL)
        xtl = xTs[b][:, sl].reshape(c.HID // 128, 128, c.SL)
        xtl = np.ascontiguousarray(
            xtl.transpose(1, 0, 2).reshape(128, -1))
        gmap = {k: v for k, v in groups[g].items() if k != "wq_part"}
        in_maps.append({
            "xT": xtl,
            "w_qb": wq_full,
            "cosA": np.ascontiguousarray(cosT[:, sl]),
            "sinA": np.ascontiguousarray(sinT[:, sl]),
            **shared, **gmap,
        })

    res = run_bass_kernel_spmd(nc, in_maps, core_ids=list(range(c.NCORES)))
    outs = [r["outT"] for r in res.results]
    result = np.empty((B, c.S, c.HID), dtype=np.float32)
    for b in range(B):
        acc = outs[b * n_groups].copy()
        for g in range(1, n_groups):
            acc += outs[b * n_groups + g]
        result[b] = acc.T
    return result
